# revision 6
# baseline (speedup 1.0000x reference)
"""Trainium2 Bass kernel for DynamicGaussianCloud (Euler integration of a
point cloud through a tiny velocity MLP, 64 steps).

Approach
--------
Data-parallel over the 8 NeuronCores: each core owns N/8 = 250k points; the
MLP weights are replicated; no cross-core communication.

On-device, instead of tracking positions pos_t (3 dims/point) we track

    state_t = W1xyz.T @ pos_t            (64 dims/point, feature-major)

where W1xyz = W1[0:3, :].  Because pos_{t+1} = pos_t + DT*(W3.T@h2 + b3 + bv),
state obeys the closed recurrence

    h1_t      = relu(state_t + b1 + t*DT*W1[3,:])        (per-partition bias)
    h2_t      = relu(W2.T @ h1_t + b2)
    state_t+1 = state_t + (DT*W3@W1xyz).T @ h2_t + W1xyz.T @ bvd    (bvd = DT*(bv+b3))

so layer 1 of the MLP disappears from the loop, and the state update is pure
PSUM accumulation by the tensor engine (start=False matmuls) — no vector-engine
adds.  Positions are recovered at the end with R = pinv(W1xyz.T):
pos_T = R @ state_T (exact in infinite precision; W1 is a random Gaussian
matrix so W1xyz.T is well-conditioned).

Two 512-point blocks are packed block-diagonally on the 128 partitions
("pair" = 1024 points).  NP pairs are kept in flight (PSUM-resident state,
one bank each); emission is stage-major so the per-engine instruction
streams interleave the NP independent dependency chains.  Per pair-step:
  ACT : 1 op  (relu + per-partition bias, PSUM->SBUF)
  PE  : 3 fp32r matmuls (free dim 512 -> 1 cycle/row)
  DVE : 1 op  (fused add-bias + max0, PSUM->SBUF)
Matmuls run in float32r: fp32 storage, ~1e-4 relative multiply precision at
full 1-cycle/row speed.  Init/final matmuls run in true fp32.
"""

import sys

sys.path.insert(0, "/opt/trn_rl_repo")

import numpy as np

import concourse.bacc as bacc
import concourse.bass as bass
import concourse.mybir as mybir
import concourse.tile as tile
from concourse.bass_utils import run_bass_kernel_spmd

f32 = mybir.dt.float32
f32r = mybir.dt.float32r

N_TOTAL = 2_000_000
DT = 1.0 / 30.0
N_CORES = 8
N_SHARD = N_TOTAL // N_CORES        # 250_000
N_STEPS = 64
W = 512                             # points per block (matmul free dim)
PAIR_PTS = 2 * W                    # 1024

NP = 4                              # pairs in flight (PSUM state banks)
WBUFS = 4                           # PSUM work banks; NP + WBUFS <= 8

AluOp = mybir.AluOpType
ActFn = mybir.ActivationFunctionType

_CACHE = {}


def _layout(np_):
    """groups per core so that G*np_ pairs cover the shard."""
    g = -(-N_SHARD // (np_ * PAIR_PTS))      # ceil
    return g, g * np_ * PAIR_PTS


def _build_nc(g_count=None, n_steps=N_STEPS, repeat=1, np_=NP, wbufs=WBUFS,
              skip_mmc=False, skip_mm13=False, stagger=False, unroll=1,
              relu1_act=4, use_bf16=True):
    """Build + compile the Bass module (shapes are static)."""
    if g_count is None:
        g_count, _ = _layout(np_)
    G = g_count
    nc = bacc.Bacc("TRN2", target_bir_lowering=False, debug=False,
                   num_devices=N_CORES)

    FD = np_ * W
    pos_d = nc.declare_dram_parameter("pos", [G * 8, FD], f32, isOutput=False)
    bvd_d = nc.declare_dram_parameter("bvd", [G * 8, FD], f32r, isOutput=False)
    w2_d = nc.declare_dram_parameter("w2blk", [128, 128], f32r, isOutput=False)
    w13_d = nc.declare_dram_parameter("w13blk", [128, 128], f32r, isOutput=False)
    wc32_d = nc.declare_dram_parameter("wc32", [8, 128], f32, isOutput=False)
    wcr_d = nc.declare_dram_parameter("wcr", [8, 128], f32r, isOutput=False)
    rt_d = nc.declare_dram_parameter("rt", [128, 8], f32, isOutput=False)
    b1t_d = nc.declare_dram_parameter("b1t", [128, N_STEPS], f32, isOutput=False)
    b2_d = nc.declare_dram_parameter("b2col", [128, 1], f32, isOutput=False)
    out_d = nc.declare_dram_parameter("out", [G * 8, FD], f32, isOutput=True)

    bf16 = mybir.dt.bfloat16
    mmdt = bf16 if use_bf16 else f32r
    with tile.TileContext(nc) as tc:
        with (
            tc.tile_pool(name="const", bufs=1) as cpool,
            tc.tile_pool(name="io", bufs=2) as iopool,
            tc.tile_pool(name="hwork", bufs=2 * np_) as hpool,
            tc.tile_pool(name="fin", bufs=np_) as fpool,
            tc.tile_pool(name="state", bufs=np_, space="PSUM") as spool,
            tc.tile_pool(name="work", bufs=wbufs, space="PSUM") as wpool,
        ):
            w2_l = cpool.tile([128, 128], f32r, tag="w2l")
            w13_l = cpool.tile([128, 128], f32r, tag="w13l")
            wcr_l = cpool.tile([8, 128], f32r, tag="wcrl")
            wc32_t = cpool.tile([8, 128], f32, tag="wc32")
            rt_t = cpool.tile([128, 8], f32, tag="rt")
            b1t_t = cpool.tile([128, N_STEPS], f32, tag="b1t")
            b2_t = cpool.tile([128, 1], f32, tag="b2")
            nc.sync.dma_start(w2_l[:], w2_d[:])
            nc.sync.dma_start(w13_l[:], w13_d[:])
            nc.sync.dma_start(wc32_t[:], wc32_d[:])
            nc.sync.dma_start(wcr_l[:], wcr_d[:])
            nc.sync.dma_start(rt_t[:], rt_d[:])
            nc.sync.dma_start(b1t_t[:], b1t_d[:])
            nc.sync.dma_start(b2_t[:], b2_d[:])
            if use_bf16:
                w2_t = cpool.tile([128, 128], mmdt, tag="w2")
                w13_t = cpool.tile([128, 128], mmdt, tag="w13")
                wcr_t = cpool.tile([8, 128], mmdt, tag="wcr")
                nc.vector.tensor_copy(w2_t[:], w2_l[:])
                nc.vector.tensor_copy(w13_t[:], w13_l[:])
                nc.vector.tensor_copy(wcr_t[:], wcr_l[:])
            else:
                w2_t, w13_t, wcr_t = w2_l, w13_l, wcr_l

            assert G % unroll == 0
            with tc.For_i(0, repeat) as _r, \
                 tc.For_i(0, G // unroll, staggered_reset=stagger) as g0:
              for u in range(unroll):
                g = g0 * unroll + u
                pos_g = iopool.tile([8, FD], f32, tag="pos")
                bvd_l = iopool.tile([8, FD], f32r, tag="bvd")
                out_g = iopool.tile([8, FD], f32, tag="out")
                nc.sync.dma_start(pos_g[:], pos_d[bass.ds(g * 8, 8), :])
                nc.sync.dma_start(bvd_l[:], bvd_d[bass.ds(g * 8, 8), :])
                if use_bf16:
                    bvd_g = iopool.tile([8, FD], mmdt, tag="bvdb")
                    nc.vector.tensor_copy(bvd_g[:], bvd_l[:])
                else:
                    bvd_g = bvd_l

                # Stage-major emission: all np_ pairs per pipeline stage, so
                # each engine's instruction stream interleaves the np_
                # independent dependency chains (Tile schedules in emission
                # order per engine).
                states = []
                for p in range(np_):
                    state = spool.tile([128, W], f32, tag="state")
                    states.append(state)
                    # state_0 = W1xyz.T @ pos (true fp32, once per pair)
                    nc.tensor.matmul(state[:], wc32_t[:], pos_g[:, bass.ts(p, W)],
                                     start=True, stop=True,
                                     skip_group_check=True)
                for t in range(n_steps):
                    h1s, ps2s, h2s = [], [], []
                    for p in range(np_):
                        h1 = hpool.tile([128, W], mmdt, tag="h1")
                        h1s.append(h1)
                        if p >= relu1_act:
                            nc.vector.tensor_scalar(h1[:], states[p][:],
                                                    b1t_t[:, t:t + 1], 0.0,
                                                    AluOp.add, AluOp.max)
                        else:
                            nc.scalar.activation(h1[:], states[p][:], ActFn.Relu,
                                                 bias=b1t_t[:, t:t + 1])
                    for p in range(np_):
                        ps2 = wpool.tile([128, W], f32, tag="work")
                        ps2s.append(ps2)
                        nc.tensor.matmul(ps2[:], w2_t[:], h1s[p][:],
                                         start=True, stop=True)
                    for p in range(np_):
                        h2 = hpool.tile([128, W], mmdt, tag="h2")
                        h2s.append(h2)
                        if p >= relu1_act:
                            nc.scalar.activation(h2[:], ps2s[p][:], ActFn.Relu,
                                                 bias=b2_t[:])
                        else:
                            nc.vector.tensor_scalar(h2[:], ps2s[p][:], b2_t[:],
                                                    0.0, AluOp.add, AluOp.max)
                    for p in range(np_):
                        if not skip_mm13:
                            nc.tensor.matmul(states[p][:], w13_t[:], h2s[p][:],
                                             start=False, stop=skip_mmc,
                                             skip_group_check=True)
                        if not skip_mmc:
                            nc.tensor.matmul(states[p][:], wcr_t[:],
                                             bvd_g[:, bass.ts(p, W)],
                                             start=False, stop=True,
                                             skip_group_check=True)
                st_sbs, pos_ = [], []
                for p in range(np_):
                    st_sb = fpool.tile([128, W], f32, tag="stsb")
                    st_sbs.append(st_sb)
                    nc.scalar.activation(st_sb[:], states[p][:], ActFn.Copy)
                for p in range(np_):
                    po = wpool.tile([8, W], f32, tag="work")
                    pos_.append(po)
                    nc.tensor.matmul(po[:], rt_t[:], st_sbs[p][:],
                                     start=True, stop=True)
                for p in range(np_):
                    nc.scalar.activation(out_g[:, bass.ts(p, W)], pos_[p][:],
                                         ActFn.Copy)

                nc.sync.dma_start(out_d[bass.ds(g * 8, 8), :], out_g[:])

    nc.compile()
    return nc


def _prep_core_inputs(pos_shard, bvel_shard, b3, np_=NP):
    """Pack one core's shard into the device layout."""
    n = pos_shard.shape[0]
    G, n_pad = _layout(np_)

    def pack(arr3):
        pad = np.zeros((n_pad, 3), np.float32)
        pad[:n] = arr3
        # [G, np_, half, W, coord=4] -> rows half*4+coord, cols p*W+w
        a4 = np.zeros((G, np_, 2, W, 4), np.float32)
        a4[..., :3] = pad.reshape(G, np_, 2, W, 3)
        return a4.transpose(0, 2, 4, 1, 3).reshape(G * 8, np_ * W).copy()

    bvd = (DT * (bvel_shard + b3[None, :])).astype(np.float32)
    return {"pos": pack(pos_shard), "bvd": pack(bvd)}


def _prep_weights(W1, b1, W2, b2, W3, b3):
    W1 = np.asarray(W1, np.float64)
    W1xyz = W1[:3]                                     # [3, 64]
    W2 = np.asarray(W2, np.float64)
    W3 = np.asarray(W3, np.float64)

    w2blk = np.zeros((128, 128), np.float32)
    w2blk[:64, :64] = W2
    w2blk[64:, 64:] = W2

    lhsT13 = DT * (W3 @ W1xyz)                         # [64, 64]
    w13blk = np.zeros((128, 128), np.float32)
    w13blk[:64, :64] = lhsT13
    w13blk[64:, 64:] = lhsT13

    wc = np.zeros((8, 128), np.float32)
    wc[0:3, :64] = W1xyz
    wc[4:7, 64:] = W1xyz

    R = np.linalg.pinv(W1xyz.T)                        # [3, 64]
    rt = np.zeros((128, 8), np.float32)
    rt[:64, 0:3] = R.T
    rt[64:, 4:7] = R.T

    t = np.arange(N_STEPS, dtype=np.float64)
    b1t_half = np.asarray(b1, np.float64)[:, None] + np.outer(W1[3], t * DT)
    b1t = np.zeros((128, N_STEPS), np.float32)
    b1t[:64] = b1t_half
    b1t[64:] = b1t_half

    b2col = np.zeros((128, 1), np.float32)
    b2col[:64, 0] = b2
    b2col[64:, 0] = b2

    return {"w2blk": w2blk, "w13blk": w13blk, "wc32": wc, "wcr": wc.copy(),
            "rt": rt, "b1t": b1t, "b2col": b2col}


def _unpack_out(packed, n, np_=NP):
    """[G*8, np_*W] device layout -> [n, 3] positions."""
    G, n_pad = _layout(np_)
    a4 = packed.reshape(G, 2, 4, np_, W).transpose(0, 3, 1, 4, 2)
    return a4[..., :3].reshape(n_pad, 3)[:n]


# ======================================================================
# V2: paired-tile kernel.
#
# PE throughput on this part is capped at ~50% average utilization (activity
# throttle) — wall-clock scales with PE streaming cycles.  V2 halves them:
# the two 64x64 matmuls per pair-step (W2 and A=DT*W3@W1xyz) are issued as
# PAIRS on complementary 64x64 PE tiles: even ("diag") pairs keep state
# block-a on partitions 0-63, odd ("anti") pairs swap the halves, so
# MM2(even pair) runs on tiles (0,0)+(64,64) concurrently with MM13(odd
# pair) on tiles (0,64)+(64,0) — 2 logical matmuls per 512-cycle window.
# The per-step constant drift (wcr) runs as 4 concurrent 32x128 row-tiles
# (one per pair) in a single window.  Relus are [128, 1024] half-quad ops
# (ACT does relu1, DVE does relu2) on 2-bank PSUM slices.
# ======================================================================

bf16 = mybir.dt.bfloat16
G2 = -(-N_SHARD // (4 * PAIR_PTS))          # 62 groups of 4 pairs
N_PAD2 = G2 * 4 * PAIR_PTS


def _build_nc2(n_steps=N_STEPS, use_bf16=True, pair_mode=True):
    G = G2
    mmdt = bf16 if use_bf16 else f32r
    nc = bacc.Bacc("TRN2", target_bir_lowering=False, debug=False,
                   num_devices=N_CORES)

    pos_d = nc.declare_dram_parameter("pos", [G * 32, W], f32, isOutput=False)
    bvd_d = nc.declare_dram_parameter("bvd", [G * 32, W],
                                      mmdt if use_bf16 else f32r, isOutput=False)
    w2d_d = nc.declare_dram_parameter("w2d", [128, 128], mmdt, isOutput=False)
    w2a_d = nc.declare_dram_parameter("w2a", [128, 128], mmdt, isOutput=False)
    w13d_d = nc.declare_dram_parameter("w13d", [128, 128], mmdt, isOutput=False)
    w13a_d = nc.declare_dram_parameter("w13a", [128, 128], mmdt, isOutput=False)
    wcr_d = nc.declare_dram_parameter("wcrblk", [128, 128], mmdt, isOutput=False)
    wc32_d = nc.declare_dram_parameter("wc32blk", [128, 128], f32, isOutput=False)
    rt_d = nc.declare_dram_parameter("rtblk", [128, 128], f32, isOutput=False)
    b1t_d = nc.declare_dram_parameter("b1t", [128, N_STEPS], f32, isOutput=False)
    b2_d = nc.declare_dram_parameter("b2col", [128, 1], f32, isOutput=False)
    out_d = nc.declare_dram_parameter("out", [G * 32, W], f32, isOutput=True)

    H = 2 * W                               # half-quad relu width (2 pairs)
    FQ = 4 * W                              # full quad width

    with tile.TileContext(nc) as tc:
        with (
            tc.tile_pool(name="const", bufs=1) as cpool,
            tc.tile_pool(name="io", bufs=2) as iopool,
            tc.tile_pool(name="hwork", bufs=2) as hpool,
            tc.tile_pool(name="fin", bufs=2) as fpool,
            tc.tile_pool(name="state", bufs=1, space="PSUM") as spool,
            tc.tile_pool(name="work", bufs=1, space="PSUM") as wpool,
        ):
            w2d_t = cpool.tile([128, 128], mmdt, tag="w2d")
            w2a_t = cpool.tile([128, 128], mmdt, tag="w2a")
            w13d_t = cpool.tile([128, 128], mmdt, tag="w13d")
            w13a_t = cpool.tile([128, 128], mmdt, tag="w13a")
            wcr_t = cpool.tile([128, 128], mmdt, tag="wcr")
            wc32_t = cpool.tile([128, 128], f32, tag="wc32")
            rt_t = cpool.tile([128, 128], f32, tag="rt")
            b1t_t = cpool.tile([128, N_STEPS], f32, tag="b1t")
            b2_t = cpool.tile([128, 1], f32, tag="b2")
            nc.sync.dma_start(w2d_t[:], w2d_d[:])
            nc.sync.dma_start(w2a_t[:], w2a_d[:])
            nc.sync.dma_start(w13d_t[:], w13d_d[:])
            nc.sync.dma_start(w13a_t[:], w13a_d[:])
            nc.sync.dma_start(wcr_t[:], wcr_d[:])
            nc.sync.dma_start(wc32_t[:], wc32_d[:])
            nc.sync.dma_start(rt_t[:], rt_d[:])
            nc.sync.dma_start(b1t_t[:], b1t_d[:])
            nc.sync.dma_start(b2_t[:], b2_d[:])

            with tc.For_i(0, G) as g:
                pos_g = iopool.tile([128, W], f32, tag="pos")
                bvd_g = iopool.tile([128, W], mmdt if use_bf16 else f32r,
                                    tag="bvd")
                out_sb = iopool.tile([128, W], f32, tag="out")
                for q in range(4):
                    nc.sync.dma_start(pos_g[32 * q:32 * q + 8, :],
                                      pos_d[bass.ds(g * 32 + q * 8, 8), :])
                    nc.sync.dma_start(bvd_g[32 * q:32 * q + 8, :],
                                      bvd_d[bass.ds(g * 32 + q * 8, 8), :])

                states = spool.tile([128, FQ], f32, tag="state")
                # init (true fp32): 4 concurrent 32x128 row-tiles
                for q in range(4):
                    nc.tensor.matmul(states[:, bass.ts(q, W)],
                                     wc32_t[32 * q:32 * q + 8, :],
                                     pos_g[32 * q:32 * q + 8, :],
                                     start=True, stop=False,
                                     tile_position=(32 * q, 0),
                                     skip_group_check=True)

                def emit_mm2(ps2, h1h, p):
                    """ps2[:, p*W:] = W2blk.T @ h1; h1h = half tile, col of p within half."""
                    c = bass.ts(p, W)
                    hc = bass.ts(p % 2, W)
                    if p % 2 == 0:      # diag tiles T0 + T10
                        nc.tensor.matmul(ps2[0:64, c], w2d_t[0:64, 0:64],
                                         h1h[0:64, hc], start=True, stop=True)
                        nc.tensor.matmul(ps2[64:128, c], w2d_t[64:128, 64:128],
                                         h1h[64:128, hc], start=True, stop=True)
                    else:               # anti tiles T8 + T2
                        nc.tensor.matmul(ps2[0:64, c], w2a_t[64:128, 0:64],
                                         h1h[64:128, hc], start=True, stop=True)
                        nc.tensor.matmul(ps2[64:128, c], w2a_t[0:64, 64:128],
                                         h1h[0:64, hc], start=True, stop=True)

                def emit_mm13(states, h2h, p, stop=False):
                    """states[:, p*W:] += A13blk.T @ h2 (h2 always normal layout)."""
                    c = bass.ts(p, W)
                    hc = bass.ts(p % 2, W)
                    if p % 2 == 0:      # diag
                        nc.tensor.matmul(states[0:64, c], w13d_t[0:64, 0:64],
                                         h2h[0:64, hc], start=False, stop=stop,
                                         skip_group_check=True)
                        nc.tensor.matmul(states[64:128, c], w13d_t[64:128, 64:128],
                                         h2h[64:128, hc], start=False, stop=stop,
                                         skip_group_check=True)
                    else:               # anti: h2 block-a (S0-63) -> state[64:128]
                        nc.tensor.matmul(states[64:128, c], w13a_t[0:64, 64:128],
                                         h2h[0:64, hc], start=False, stop=stop,
                                         skip_group_check=True)
                        nc.tensor.matmul(states[0:64, c], w13a_t[64:128, 0:64],
                                         h2h[64:128, hc], start=False, stop=stop,
                                         skip_group_check=True)

                h2_prev = None          # h2 half-tile of H1 from previous step
                for t in range(n_steps):
                    # ---- half H0 (pairs 0, 1) ----
                    h1_0 = hpool.tile([128, H], mmdt, tag="h1_0")
                    nc.scalar.activation(h1_0[:], states[:, 0:H], ActFn.Relu,
                                         bias=b1t_t[:, t:t + 1])
                    ps2 = wpool.tile([128, FQ], f32, tag="ps2")
                    # W1: MM2-p0 (diag) || MM13-p3(t-1) (anti)
                    emit_mm2(ps2, h1_0, 0)
                    if h2_prev is not None:
                        emit_mm13(states, h2_prev, 3)
                    # W2: MM2-p1 (anti) || MM13-p2(t-1) (diag)
                    emit_mm2(ps2, h1_0, 1)
                    if h2_prev is not None:
                        emit_mm13(states, h2_prev, 2)
                    # relu2-H0 on ACT: keeps the critical chain
                    # (relu1-H0 -> W1/W2 -> relu2-H0 -> W3/W4 -> wcr) on one
                    # fast engine; H1's relus go to DVE off-chain.
                    h2_0 = hpool.tile([128, H], mmdt, tag="h2_0")
                    nc.scalar.activation(h2_0[:], ps2[:, 0:H], ActFn.Relu,
                                         bias=b2_t[:])
                    # ---- half H1 (pairs 2, 3) ----
                    h1_1 = hpool.tile([128, H], mmdt, tag="h1_1")
                    nc.vector.tensor_scalar(h1_1[:], states[:, H:FQ],
                                            b1t_t[:, t:t + 1], 0.0,
                                            AluOp.add, AluOp.max)
                    # W3: MM2-p2 (diag) || MM13-p1(t) (anti)
                    emit_mm2(ps2, h1_1, 2)
                    emit_mm13(states, h2_0, 1)
                    # W4: MM2-p3 (anti) || MM13-p0(t) (diag)
                    emit_mm2(ps2, h1_1, 3)
                    emit_mm13(states, h2_0, 0)
                    h2_1 = hpool.tile([128, H], mmdt, tag="h2_1")
                    nc.vector.tensor_scalar(h2_1[:], ps2[:, H:FQ], b2_t[:], 0.0,
                                            AluOp.add, AluOp.max)
                    # ---- wcr window: 4 concurrent 32x128 row-tiles ----
                    for q in range(4):
                        nc.tensor.matmul(states[:, bass.ts(q, W)],
                                         wcr_t[32 * q:32 * q + 8, :],
                                         bvd_g[32 * q:32 * q + 8, :],
                                         start=False, stop=True,
                                         tile_position=(32 * q, 0),
                                         skip_group_check=True)
                    h2_prev = h2_1

                # tail: MM13-p2(T-1) (diag) || MM13-p3(T-1) (anti)
                emit_mm13(states, h2_prev, 2, stop=True)
                emit_mm13(states, h2_prev, 3, stop=True)

                # finale
                st_sb = fpool.tile([128, FQ], f32, tag="stsb")
                nc.scalar.activation(st_sb[:], states[:], ActFn.Copy)
                out_ps = wpool.tile([128, FQ], f32, tag="ps2")
                for p in range(4):
                    nc.tensor.matmul(out_ps[32 * p:32 * p + 8, 0:W],
                                     rt_t[:, 32 * p:32 * p + 8],
                                     st_sb[:, bass.ts(p, W)],
                                     start=True, stop=True,
                                     tile_position=(0, 32 * p),
                                     skip_group_check=True)
                nc.scalar.activation(out_sb[:], out_ps[:, 0:W], ActFn.Copy)
                for q in range(4):
                    nc.sync.dma_start(out_d[bass.ds(g * 32 + q * 8, 8), :],
                                      out_sb[32 * q:32 * q + 8, :])

    nc.compile()
    return nc


def _pack2(arr3, n):
    """[n,3] -> [G2*32, W]: row = g*32 + q*8 + half*4 + coord, col = point."""
    pad = np.zeros((N_PAD2, 3), np.float32)
    pad[:n] = arr3
    a5 = pad.reshape(G2, 4, 2, W, 3)              # g, q, half, w, coord
    buf = np.zeros((G2, 4, 2, 4, W), np.float32)  # g, q, half, coord, w
    buf[:, :, :, :3, :] = a5.transpose(0, 1, 2, 4, 3)
    return buf.reshape(G2 * 32, W)


def _unpack2(packed, n):
    a5 = packed.reshape(G2, 4, 2, 4, W)[:, :, :, :3, :]
    return a5.transpose(0, 1, 2, 4, 3).reshape(N_PAD2, 3)[:n].copy()


def _prep_core_inputs2(pos_shard, bvel_shard, b3, use_bf16=True):
    n = pos_shard.shape[0]
    bvd = (DT * (bvel_shard + b3[None, :])).astype(np.float32)
    bvd_p = _pack2(bvd, n)
    if use_bf16:
        bvd_p = bvd_p.astype(mybir.dt.np(bf16))
    return {"pos": _pack2(pos_shard, n), "bvd": bvd_p}


def _prep_weights2(W1, b1, W2, b2, W3, b3, use_bf16=True):
    W1 = np.asarray(W1, np.float64)
    W1xyz = W1[:3]                                     # [3, 64]
    W2 = np.asarray(W2, np.float64)
    W3 = np.asarray(W3, np.float64)
    A13 = DT * (W3 @ W1xyz)                            # [64, 64] lhsT

    def blk(mat, anti):
        out = np.zeros((128, 128), np.float32)
        if anti:
            out[0:64, 64:128] = mat
            out[64:128, 0:64] = mat
        else:
            out[0:64, 0:64] = mat
            out[64:128, 64:128] = mat
        return out

    # wcr / wc32: rows 32q+{0..2} and 32q+{4..6}; odd q swaps column halves
    wcr = np.zeros((128, 128), np.float32)
    for q in range(4):
        lo, hi = (slice(0, 64), slice(64, 128))
        if q % 2 == 1:
            lo, hi = hi, lo
        wcr[32 * q + 0:32 * q + 3, lo] = W1xyz
        wcr[32 * q + 4:32 * q + 7, hi] = W1xyz

    R = np.linalg.pinv(W1xyz.T)                        # [3, 64]
    rt = np.zeros((128, 128), np.float32)
    for p in range(4):
        lo, hi = (slice(0, 64), slice(64, 128))
        if p % 2 == 1:
            lo, hi = hi, lo
        rt[lo, 32 * p + 0:32 * p + 3] = R.T
        rt[hi, 32 * p + 4:32 * p + 7] = R.T

    t = np.arange(N_STEPS, dtype=np.float64)
    b1t_half = np.asarray(b1, np.float64)[:, None] + np.outer(W1[3], t * DT)
    b1t = np.zeros((128, N_STEPS), np.float32)
    b1t[:64] = b1t_half
    b1t[64:] = b1t_half

    b2col = np.zeros((128, 1), np.float32)
    b2col[:64, 0] = b2
    b2col[64:, 0] = b2

    wmap = {"w2d": blk(W2, False), "w2a": blk(W2, True),
            "w13d": blk(A13, False), "w13a": blk(A13, True),
            "wcrblk": wcr, "wc32blk": wcr.copy(), "rtblk": rt,
            "b1t": b1t, "b2col": b2col}
    if use_bf16:
        npbf = mybir.dt.np(bf16)
        for k in ("w2d", "w2a", "w13d", "w13a", "wcrblk"):
            wmap[k] = wmap[k].astype(npbf)
    return wmap


V2 = True


def _kernel_v2(positions, base_velocities, W1, b1, W2, b2, W3, b3, n_steps):
    if "nc2" not in _CACHE:
        _CACHE["nc2"] = _build_nc2()
    nc = _CACHE["nc2"]

    b3 = np.asarray(b3, np.float32)
    wmap = _prep_weights2(W1, b1, W2, b2, W3, b3)
    in_maps = []
    for c in range(N_CORES):
        sl = slice(c * N_SHARD, (c + 1) * N_SHARD)
        m = _prep_core_inputs2(positions[sl], base_velocities[sl], b3)
        m.update(wmap)
        in_maps.append(m)

    res = run_bass_kernel_spmd(nc, in_maps, core_ids=list(range(N_CORES)))

    out = np.empty((N_TOTAL, 3), np.float32)
    for c in range(N_CORES):
        out[c * N_SHARD:(c + 1) * N_SHARD] = _unpack2(
            res.results[c]["out"], N_SHARD)
    return out


def kernel(positions, base_velocities, W1, b1, W2, b2, W3, b3, n_steps):
    assert int(n_steps) == N_STEPS
    positions = np.asarray(positions, np.float32)
    base_velocities = np.asarray(base_velocities, np.float32)
    if V2:
        return _kernel_v2(positions, base_velocities, W1, b1, W2, b2, W3, b3,
                          n_steps)
    W1 = np.asarray(W1, np.float32)
    b1 = np.asarray(b1, np.float32)
    W2 = np.asarray(W2, np.float32)
    b2 = np.asarray(b2, np.float32)
    W3 = np.asarray(W3, np.float32)
    b3 = np.asarray(b3, np.float32)

    if "nc" not in _CACHE:
        _CACHE["nc"] = _build_nc()
    nc = _CACHE["nc"]

    wmap = _prep_weights(W1, b1, W2, b2, W3, b3)
    in_maps = []
    for c in range(N_CORES):
        sl = slice(c * N_SHARD, (c + 1) * N_SHARD)
        m = _prep_core_inputs(positions[sl], base_velocities[sl], b3)
        m.update(wmap)
        in_maps.append(m)

    res = run_bass_kernel_spmd(nc, in_maps, core_ids=list(range(N_CORES)))

    out = np.empty((N_TOTAL, 3), np.float32)
    for c in range(N_CORES):
        out[c * N_SHARD:(c + 1) * N_SHARD] = _unpack_out(
            res.results[c]["out"], N_SHARD)
    return out



# revision 14
# speedup vs baseline: 1.1513x; 1.1513x over previous
"""Trainium2 Bass kernel for DynamicGaussianCloud (Euler integration of a
point cloud through a tiny velocity MLP, 64 steps).

The active implementation is V2 (`_build_nc2`, selected via V2=True below):
bf16 matmuls + paired 64x64 PE-tile windows + half-quad relus; see the V2
section banner for details.  The original f32r kernel (`_build_nc`) is kept
for reference/AB-testing.

V1 approach
-----------
Data-parallel over the 8 NeuronCores: each core owns N/8 = 250k points; the
MLP weights are replicated; no cross-core communication.

On-device, instead of tracking positions pos_t (3 dims/point) we track

    state_t = W1xyz.T @ pos_t            (64 dims/point, feature-major)

where W1xyz = W1[0:3, :].  Because pos_{t+1} = pos_t + DT*(W3.T@h2 + b3 + bv),
state obeys the closed recurrence

    h1_t      = relu(state_t + b1 + t*DT*W1[3,:])        (per-partition bias)
    h2_t      = relu(W2.T @ h1_t + b2)
    state_t+1 = state_t + (DT*W3@W1xyz).T @ h2_t + W1xyz.T @ bvd    (bvd = DT*(bv+b3))

so layer 1 of the MLP disappears from the loop, and the state update is pure
PSUM accumulation by the tensor engine (start=False matmuls) — no vector-engine
adds.  Positions are recovered at the end with R = pinv(W1xyz.T):
pos_T = R @ state_T (exact in infinite precision; W1 is a random Gaussian
matrix so W1xyz.T is well-conditioned).

Two 512-point blocks are packed block-diagonally on the 128 partitions
("pair" = 1024 points).  NP pairs are kept in flight (PSUM-resident state,
one bank each); emission is stage-major so the per-engine instruction
streams interleave the NP independent dependency chains.  Per pair-step:
  ACT : 1 op  (relu + per-partition bias, PSUM->SBUF)
  PE  : 3 fp32r matmuls (free dim 512 -> 1 cycle/row)
  DVE : 1 op  (fused add-bias + max0, PSUM->SBUF)
Matmuls run in float32r: fp32 storage, ~1e-4 relative multiply precision at
full 1-cycle/row speed.  Init/final matmuls run in true fp32.
"""

import sys

sys.path.insert(0, "/opt/trn_rl_repo")

import numpy as np

import concourse.bacc as bacc
import concourse.bass as bass
import concourse.mybir as mybir
import concourse.tile as tile
from concourse.bass_utils import run_bass_kernel_spmd

f32 = mybir.dt.float32
f32r = mybir.dt.float32r

N_TOTAL = 2_000_000
DT = 1.0 / 30.0
N_CORES = 8
N_SHARD = N_TOTAL // N_CORES        # 250_000
N_STEPS = 64
W = 512                             # points per block (matmul free dim)
PAIR_PTS = 2 * W                    # 1024

NP = 4                              # pairs in flight (PSUM state banks)
WBUFS = 4                           # PSUM work banks; NP + WBUFS <= 8

AluOp = mybir.AluOpType
ActFn = mybir.ActivationFunctionType

_CACHE = {}


def _layout(np_):
    """groups per core so that G*np_ pairs cover the shard."""
    g = -(-N_SHARD // (np_ * PAIR_PTS))      # ceil
    return g, g * np_ * PAIR_PTS


def _build_nc(g_count=None, n_steps=N_STEPS, repeat=1, np_=NP, wbufs=WBUFS,
              skip_mmc=False, skip_mm13=False, stagger=False, unroll=1,
              relu1_act=4, use_bf16=True):
    """Build + compile the Bass module (shapes are static)."""
    if g_count is None:
        g_count, _ = _layout(np_)
    G = g_count
    nc = bacc.Bacc("TRN2", target_bir_lowering=False, debug=False,
                   num_devices=N_CORES)

    FD = np_ * W
    pos_d = nc.declare_dram_parameter("pos", [G * 8, FD], f32, isOutput=False)
    bvd_d = nc.declare_dram_parameter("bvd", [G * 8, FD], f32r, isOutput=False)
    w2_d = nc.declare_dram_parameter("w2blk", [128, 128], f32r, isOutput=False)
    w13_d = nc.declare_dram_parameter("w13blk", [128, 128], f32r, isOutput=False)
    wc32_d = nc.declare_dram_parameter("wc32", [8, 128], f32, isOutput=False)
    wcr_d = nc.declare_dram_parameter("wcr", [8, 128], f32r, isOutput=False)
    rt_d = nc.declare_dram_parameter("rt", [128, 8], f32, isOutput=False)
    b1t_d = nc.declare_dram_parameter("b1t", [128, N_STEPS], f32, isOutput=False)
    b2_d = nc.declare_dram_parameter("b2col", [128, 1], f32, isOutput=False)
    out_d = nc.declare_dram_parameter("out", [G * 8, FD], f32, isOutput=True)

    bf16 = mybir.dt.bfloat16
    mmdt = bf16 if use_bf16 else f32r
    with tile.TileContext(nc) as tc:
        with (
            tc.tile_pool(name="const", bufs=1) as cpool,
            tc.tile_pool(name="io", bufs=2) as iopool,
            tc.tile_pool(name="hwork", bufs=2 * np_) as hpool,
            tc.tile_pool(name="fin", bufs=np_) as fpool,
            tc.tile_pool(name="state", bufs=np_, space="PSUM") as spool,
            tc.tile_pool(name="work", bufs=wbufs, space="PSUM") as wpool,
        ):
            w2_l = cpool.tile([128, 128], f32r, tag="w2l")
            w13_l = cpool.tile([128, 128], f32r, tag="w13l")
            wcr_l = cpool.tile([8, 128], f32r, tag="wcrl")
            wc32_t = cpool.tile([8, 128], f32, tag="wc32")
            rt_t = cpool.tile([128, 8], f32, tag="rt")
            b1t_t = cpool.tile([128, N_STEPS], f32, tag="b1t")
            b2_t = cpool.tile([128, 1], f32, tag="b2")
            nc.sync.dma_start(w2_l[:], w2_d[:])
            nc.sync.dma_start(w13_l[:], w13_d[:])
            nc.sync.dma_start(wc32_t[:], wc32_d[:])
            nc.sync.dma_start(wcr_l[:], wcr_d[:])
            nc.sync.dma_start(rt_t[:], rt_d[:])
            nc.sync.dma_start(b1t_t[:], b1t_d[:])
            nc.sync.dma_start(b2_t[:], b2_d[:])
            if use_bf16:
                w2_t = cpool.tile([128, 128], mmdt, tag="w2")
                w13_t = cpool.tile([128, 128], mmdt, tag="w13")
                wcr_t = cpool.tile([8, 128], mmdt, tag="wcr")
                nc.vector.tensor_copy(w2_t[:], w2_l[:])
                nc.vector.tensor_copy(w13_t[:], w13_l[:])
                nc.vector.tensor_copy(wcr_t[:], wcr_l[:])
            else:
                w2_t, w13_t, wcr_t = w2_l, w13_l, wcr_l

            assert G % unroll == 0
            with tc.For_i(0, repeat) as _r, \
                 tc.For_i(0, G // unroll, staggered_reset=stagger) as g0:
              for u in range(unroll):
                g = g0 * unroll + u
                pos_g = iopool.tile([8, FD], f32, tag="pos")
                bvd_l = iopool.tile([8, FD], f32r, tag="bvd")
                out_g = iopool.tile([8, FD], f32, tag="out")
                nc.sync.dma_start(pos_g[:], pos_d[bass.ds(g * 8, 8), :])
                nc.sync.dma_start(bvd_l[:], bvd_d[bass.ds(g * 8, 8), :])
                if use_bf16:
                    bvd_g = iopool.tile([8, FD], mmdt, tag="bvdb")
                    nc.vector.tensor_copy(bvd_g[:], bvd_l[:])
                else:
                    bvd_g = bvd_l

                # Stage-major emission: all np_ pairs per pipeline stage, so
                # each engine's instruction stream interleaves the np_
                # independent dependency chains (Tile schedules in emission
                # order per engine).
                states = []
                for p in range(np_):
                    state = spool.tile([128, W], f32, tag="state")
                    states.append(state)
                    # state_0 = W1xyz.T @ pos (true fp32, once per pair)
                    nc.tensor.matmul(state[:], wc32_t[:], pos_g[:, bass.ts(p, W)],
                                     start=True, stop=True,
                                     skip_group_check=True)
                for t in range(n_steps):
                    h1s, ps2s, h2s = [], [], []
                    for p in range(np_):
                        h1 = hpool.tile([128, W], mmdt, tag="h1")
                        h1s.append(h1)
                        if p >= relu1_act:
                            nc.vector.tensor_scalar(h1[:], states[p][:],
                                                    b1t_t[:, t:t + 1], 0.0,
                                                    AluOp.add, AluOp.max)
                        else:
                            nc.scalar.activation(h1[:], states[p][:], ActFn.Relu,
                                                 bias=b1t_t[:, t:t + 1])
                    for p in range(np_):
                        ps2 = wpool.tile([128, W], f32, tag="work")
                        ps2s.append(ps2)
                        nc.tensor.matmul(ps2[:], w2_t[:], h1s[p][:],
                                         start=True, stop=True)
                    for p in range(np_):
                        h2 = hpool.tile([128, W], mmdt, tag="h2")
                        h2s.append(h2)
                        if p >= relu1_act:
                            nc.scalar.activation(h2[:], ps2s[p][:], ActFn.Relu,
                                                 bias=b2_t[:])
                        else:
                            nc.vector.tensor_scalar(h2[:], ps2s[p][:], b2_t[:],
                                                    0.0, AluOp.add, AluOp.max)
                    for p in range(np_):
                        if not skip_mm13:
                            nc.tensor.matmul(states[p][:], w13_t[:], h2s[p][:],
                                             start=False, stop=skip_mmc,
                                             skip_group_check=True)
                        if not skip_mmc:
                            nc.tensor.matmul(states[p][:], wcr_t[:],
                                             bvd_g[:, bass.ts(p, W)],
                                             start=False, stop=True,
                                             skip_group_check=True)
                st_sbs, pos_ = [], []
                for p in range(np_):
                    st_sb = fpool.tile([128, W], f32, tag="stsb")
                    st_sbs.append(st_sb)
                    nc.scalar.activation(st_sb[:], states[p][:], ActFn.Copy)
                for p in range(np_):
                    po = wpool.tile([8, W], f32, tag="work")
                    pos_.append(po)
                    nc.tensor.matmul(po[:], rt_t[:], st_sbs[p][:],
                                     start=True, stop=True)
                for p in range(np_):
                    nc.scalar.activation(out_g[:, bass.ts(p, W)], pos_[p][:],
                                         ActFn.Copy)

                nc.sync.dma_start(out_d[bass.ds(g * 8, 8), :], out_g[:])

    nc.compile()
    return nc


def _prep_core_inputs(pos_shard, bvel_shard, b3, np_=NP):
    """Pack one core's shard into the device layout."""
    n = pos_shard.shape[0]
    G, n_pad = _layout(np_)

    def pack(arr3):
        pad = np.zeros((n_pad, 3), np.float32)
        pad[:n] = arr3
        # [G, np_, half, W, coord=4] -> rows half*4+coord, cols p*W+w
        a4 = np.zeros((G, np_, 2, W, 4), np.float32)
        a4[..., :3] = pad.reshape(G, np_, 2, W, 3)
        return a4.transpose(0, 2, 4, 1, 3).reshape(G * 8, np_ * W).copy()

    bvd = (DT * (bvel_shard + b3[None, :])).astype(np.float32)
    return {"pos": pack(pos_shard), "bvd": pack(bvd)}


def _prep_weights(W1, b1, W2, b2, W3, b3):
    W1 = np.asarray(W1, np.float64)
    W1xyz = W1[:3]                                     # [3, 64]
    W2 = np.asarray(W2, np.float64)
    W3 = np.asarray(W3, np.float64)

    w2blk = np.zeros((128, 128), np.float32)
    w2blk[:64, :64] = W2
    w2blk[64:, 64:] = W2

    lhsT13 = DT * (W3 @ W1xyz)                         # [64, 64]
    w13blk = np.zeros((128, 128), np.float32)
    w13blk[:64, :64] = lhsT13
    w13blk[64:, 64:] = lhsT13

    wc = np.zeros((8, 128), np.float32)
    wc[0:3, :64] = W1xyz
    wc[4:7, 64:] = W1xyz

    R = np.linalg.pinv(W1xyz.T)                        # [3, 64]
    rt = np.zeros((128, 8), np.float32)
    rt[:64, 0:3] = R.T
    rt[64:, 4:7] = R.T

    t = np.arange(N_STEPS, dtype=np.float64)
    b1t_half = np.asarray(b1, np.float64)[:, None] + np.outer(W1[3], t * DT)
    b1t = np.zeros((128, N_STEPS), np.float32)
    b1t[:64] = b1t_half
    b1t[64:] = b1t_half

    b2col = np.zeros((128, 1), np.float32)
    b2col[:64, 0] = b2
    b2col[64:, 0] = b2

    return {"w2blk": w2blk, "w13blk": w13blk, "wc32": wc, "wcr": wc.copy(),
            "rt": rt, "b1t": b1t, "b2col": b2col}


def _unpack_out(packed, n, np_=NP):
    """[G*8, np_*W] device layout -> [n, 3] positions."""
    G, n_pad = _layout(np_)
    a4 = packed.reshape(G, 2, 4, np_, W).transpose(0, 3, 1, 4, 2)
    return a4[..., :3].reshape(n_pad, 3)[:n]


# ======================================================================
# V2: paired-tile kernel.
#
# PE throughput on this part is capped at ~50% average utilization (activity
# throttle) — wall-clock scales with PE streaming cycles.  V2 halves them:
# the two 64x64 matmuls per pair-step (W2 and A=DT*W3@W1xyz) are issued as
# PAIRS on complementary 64x64 PE tiles: even ("diag") pairs keep state
# block-a on partitions 0-63, odd ("anti") pairs swap the halves, so
# MM2(even pair) runs on tiles (0,0)+(64,64) concurrently with MM13(odd
# pair) on tiles (0,64)+(64,0) — 2 logical matmuls per 512-cycle window.
# The per-step constant drift (wcr) runs as 4 concurrent 32x128 row-tiles
# (one per pair) in a single window.  Relus are [128, 1024] half-quad ops
# (ACT does relu1, DVE does relu2) on 2-bank PSUM slices.
# ======================================================================

bf16 = mybir.dt.bfloat16
G2 = -(-N_SHARD // (4 * PAIR_PTS))          # 62 groups of 4 pairs
N_PAD2 = G2 * 4 * PAIR_PTS


def _build_nc2(n_steps=N_STEPS, use_bf16=True, pair_mode=True):
    G = G2
    mmdt = bf16 if use_bf16 else f32r
    nc = bacc.Bacc("TRN2", target_bir_lowering=False, debug=False,
                   num_devices=N_CORES)

    pos_d = nc.declare_dram_parameter("pos", [G * 32, W], f32, isOutput=False)
    bvd_d = nc.declare_dram_parameter("bvd", [G * 32, W],
                                      mmdt if use_bf16 else f32r, isOutput=False)
    w2d_d = nc.declare_dram_parameter("w2d", [128, 128], mmdt, isOutput=False)
    w2a_d = nc.declare_dram_parameter("w2a", [128, 128], mmdt, isOutput=False)
    w13d_d = nc.declare_dram_parameter("w13d", [128, 128], mmdt, isOutput=False)
    w13a_d = nc.declare_dram_parameter("w13a", [128, 128], mmdt, isOutput=False)
    wcr_d = nc.declare_dram_parameter("wcrblk", [128, 128], mmdt, isOutput=False)
    wc32_d = nc.declare_dram_parameter("wc32blk", [128, 128], f32, isOutput=False)
    rt_d = nc.declare_dram_parameter("rtblk", [128, 128], f32, isOutput=False)
    b1t_d = nc.declare_dram_parameter("b1t", [128, N_STEPS], f32, isOutput=False)
    b2_d = nc.declare_dram_parameter("b2col", [128, 1], f32, isOutput=False)
    out_d = nc.declare_dram_parameter("out", [G * 32, W], f32, isOutput=True)

    H = 2 * W                               # half-quad relu width (2 pairs)
    FQ = 4 * W                              # full quad width

    with tile.TileContext(nc) as tc:
        with (
            tc.tile_pool(name="const", bufs=1) as cpool,
            tc.tile_pool(name="io", bufs=2) as iopool,
            tc.tile_pool(name="hwork", bufs=2) as hpool,
            tc.tile_pool(name="fin", bufs=2) as fpool,
            tc.tile_pool(name="state", bufs=1, space="PSUM") as spool,
            tc.tile_pool(name="work", bufs=1, space="PSUM") as wpool,
        ):
            w2d_t = cpool.tile([128, 128], mmdt, tag="w2d")
            w2a_t = cpool.tile([128, 128], mmdt, tag="w2a")
            w13d_t = cpool.tile([128, 128], mmdt, tag="w13d")
            w13a_t = cpool.tile([128, 128], mmdt, tag="w13a")
            wcr_t = cpool.tile([128, 128], mmdt, tag="wcr")
            wc32_t = cpool.tile([128, 128], f32, tag="wc32")
            rt_t = cpool.tile([128, 128], f32, tag="rt")
            b1t_t = cpool.tile([128, N_STEPS], f32, tag="b1t")
            b2_t = cpool.tile([128, 1], f32, tag="b2")
            nc.sync.dma_start(w2d_t[:], w2d_d[:])
            nc.sync.dma_start(w2a_t[:], w2a_d[:])
            nc.sync.dma_start(w13d_t[:], w13d_d[:])
            nc.sync.dma_start(w13a_t[:], w13a_d[:])
            nc.sync.dma_start(wcr_t[:], wcr_d[:])
            nc.sync.dma_start(wc32_t[:], wc32_d[:])
            nc.sync.dma_start(rt_t[:], rt_d[:])
            nc.sync.dma_start(b1t_t[:], b1t_d[:])
            nc.sync.dma_start(b2_t[:], b2_d[:])

            with tc.For_i(0, G) as g:
                pos_g = iopool.tile([128, W], f32, tag="pos")
                bvd_g = iopool.tile([128, W], mmdt if use_bf16 else f32r,
                                    tag="bvd")
                out_sb = iopool.tile([128, W], f32, tag="out")
                for q in range(4):
                    nc.sync.dma_start(pos_g[32 * q:32 * q + 8, :],
                                      pos_d[bass.ds(g * 32 + q * 8, 8), :])
                    nc.sync.dma_start(bvd_g[32 * q:32 * q + 8, :],
                                      bvd_d[bass.ds(g * 32 + q * 8, 8), :])

                states = spool.tile([128, FQ], f32, tag="state")
                # init (true fp32): 4 concurrent 32x128 row-tiles
                for q in range(4):
                    nc.tensor.matmul(states[:, bass.ts(q, W)],
                                     wc32_t[32 * q:32 * q + 8, :],
                                     pos_g[32 * q:32 * q + 8, :],
                                     start=True, stop=False,
                                     tile_position=(32 * q, 0),
                                     skip_group_check=True)

                def emit_mm2(ps2, h1h, p):
                    """ps2[:, p*W:] = W2blk.T @ h1; h1h = half tile, col of p within half."""
                    c = bass.ts(p, W)
                    hc = bass.ts(p % 2, W)
                    if p % 2 == 0:      # diag tiles T0 + T10
                        nc.tensor.matmul(ps2[0:64, c], w2d_t[0:64, 0:64],
                                         h1h[0:64, hc], start=True, stop=True)
                        nc.tensor.matmul(ps2[64:128, c], w2d_t[64:128, 64:128],
                                         h1h[64:128, hc], start=True, stop=True)
                    else:               # anti tiles T8 + T2
                        nc.tensor.matmul(ps2[0:64, c], w2a_t[64:128, 0:64],
                                         h1h[64:128, hc], start=True, stop=True)
                        nc.tensor.matmul(ps2[64:128, c], w2a_t[0:64, 64:128],
                                         h1h[0:64, hc], start=True, stop=True)

                def emit_mm13(states, h2h, p, stop=False):
                    """states[:, p*W:] += A13blk.T @ h2 (h2 always normal layout)."""
                    c = bass.ts(p, W)
                    hc = bass.ts(p % 2, W)
                    if p % 2 == 0:      # diag
                        nc.tensor.matmul(states[0:64, c], w13d_t[0:64, 0:64],
                                         h2h[0:64, hc], start=False, stop=stop,
                                         skip_group_check=True)
                        nc.tensor.matmul(states[64:128, c], w13d_t[64:128, 64:128],
                                         h2h[64:128, hc], start=False, stop=stop,
                                         skip_group_check=True)
                    else:               # anti: h2 block-a (S0-63) -> state[64:128]
                        nc.tensor.matmul(states[64:128, c], w13a_t[0:64, 64:128],
                                         h2h[0:64, hc], start=False, stop=stop,
                                         skip_group_check=True)
                        nc.tensor.matmul(states[0:64, c], w13a_t[64:128, 0:64],
                                         h2h[64:128, hc], start=False, stop=stop,
                                         skip_group_check=True)

                h2_prev = None          # h2 half-tile of H1 from previous step
                for t in range(n_steps):
                    # ---- half H0 (pairs 0, 1) ----
                    h1_0 = hpool.tile([128, H], mmdt, tag="h1_0")
                    nc.scalar.activation(h1_0[:], states[:, 0:H], ActFn.Relu,
                                         bias=b1t_t[:, t:t + 1])
                    ps2 = wpool.tile([128, FQ], f32, tag="ps2")
                    # W1: MM2-p0 (diag) || MM13-p3(t-1) (anti)
                    emit_mm2(ps2, h1_0, 0)
                    if h2_prev is not None:
                        emit_mm13(states, h2_prev, 3)
                    # W2: MM2-p1 (anti) || MM13-p2(t-1) (diag)
                    emit_mm2(ps2, h1_0, 1)
                    if h2_prev is not None:
                        emit_mm13(states, h2_prev, 2)
                    # wcr for pairs 0/1 here: only gated by bvd + the WAR on
                    # relu1-H0(t), so the PE streams these row-tiles while
                    # relu2-H0 runs on DVE — off the critical chain (the
                    # chain-tail wcr segment gated relu1-H0(t+1) before).
                    for q in range(2):
                        nc.tensor.matmul(states[:, bass.ts(q, W)],
                                         wcr_t[32 * q:32 * q + 8, :],
                                         bvd_g[32 * q:32 * q + 8, :],
                                         start=False, stop=True,
                                         tile_position=(32 * q, 0),
                                         skip_group_check=True)
                    h2_0 = hpool.tile([128, H], mmdt, tag="h2_0")
                    nc.vector.tensor_scalar(h2_0[:], ps2[:, 0:H], b2_t[:], 0.0,
                                            AluOp.add, AluOp.max)
                    # ---- half H1 (pairs 2, 3) ----
                    h1_1 = hpool.tile([128, H], mmdt, tag="h1_1")
                    nc.scalar.activation(h1_1[:], states[:, H:FQ], ActFn.Relu,
                                         bias=b1t_t[:, t:t + 1])
                    # W3: MM2-p2 (diag) || MM13-p1(t) (anti)
                    emit_mm2(ps2, h1_1, 2)
                    emit_mm13(states, h2_0, 1)
                    # W4: MM2-p3 (anti) || MM13-p0(t) (diag)
                    emit_mm2(ps2, h1_1, 3)
                    emit_mm13(states, h2_0, 0)
                    # wcr for pairs 2/3 (gated by the relu1-H1(t) WAR): runs
                    # while relu2-H1 is on DVE.
                    for q in range(2, 4):
                        nc.tensor.matmul(states[:, bass.ts(q, W)],
                                         wcr_t[32 * q:32 * q + 8, :],
                                         bvd_g[32 * q:32 * q + 8, :],
                                         start=False, stop=True,
                                         tile_position=(32 * q, 0),
                                         skip_group_check=True)
                    h2_1 = hpool.tile([128, H], mmdt, tag="h2_1")
                    nc.vector.tensor_scalar(h2_1[:], ps2[:, H:FQ], b2_t[:], 0.0,
                                            AluOp.add, AluOp.max)
                    h2_prev = h2_1

                # tail: MM13-p2(T-1) (diag) || MM13-p3(T-1) (anti)
                emit_mm13(states, h2_prev, 2, stop=True)
                emit_mm13(states, h2_prev, 3, stop=True)

                # finale
                st_sb = fpool.tile([128, FQ], f32, tag="stsb")
                nc.scalar.activation(st_sb[:], states[:], ActFn.Copy)
                out_ps = wpool.tile([128, FQ], f32, tag="ps2")
                for p in range(4):
                    nc.tensor.matmul(out_ps[32 * p:32 * p + 8, 0:W],
                                     rt_t[:, 32 * p:32 * p + 8],
                                     st_sb[:, bass.ts(p, W)],
                                     start=True, stop=True,
                                     tile_position=(0, 32 * p),
                                     skip_group_check=True)
                nc.scalar.activation(out_sb[:], out_ps[:, 0:W], ActFn.Copy)
                for q in range(4):
                    nc.sync.dma_start(out_d[bass.ds(g * 32 + q * 8, 8), :],
                                      out_sb[32 * q:32 * q + 8, :])

    nc.compile()
    return nc


def _pack2(arr3, n):
    """[n,3] -> [G2*32, W]: row = g*32 + q*8 + half*4 + coord, col = point."""
    pad = np.zeros((N_PAD2, 3), np.float32)
    pad[:n] = arr3
    a5 = pad.reshape(G2, 4, 2, W, 3)              # g, q, half, w, coord
    buf = np.zeros((G2, 4, 2, 4, W), np.float32)  # g, q, half, coord, w
    buf[:, :, :, :3, :] = a5.transpose(0, 1, 2, 4, 3)
    return buf.reshape(G2 * 32, W)


def _unpack2(packed, n):
    a5 = packed.reshape(G2, 4, 2, 4, W)[:, :, :, :3, :]
    return a5.transpose(0, 1, 2, 4, 3).reshape(N_PAD2, 3)[:n].copy()


def _prep_core_inputs2(pos_shard, bvel_shard, b3, use_bf16=True):
    n = pos_shard.shape[0]
    bvd = (DT * (bvel_shard + b3[None, :])).astype(np.float32)
    bvd_p = _pack2(bvd, n)
    if use_bf16:
        bvd_p = bvd_p.astype(mybir.dt.np(bf16))
    return {"pos": _pack2(pos_shard, n), "bvd": bvd_p}


def _prep_weights2(W1, b1, W2, b2, W3, b3, use_bf16=True):
    W1 = np.asarray(W1, np.float64)
    W1xyz = W1[:3]                                     # [3, 64]
    W2 = np.asarray(W2, np.float64)
    W3 = np.asarray(W3, np.float64)
    A13 = DT * (W3 @ W1xyz)                            # [64, 64] lhsT

    def blk(mat, anti):
        out = np.zeros((128, 128), np.float32)
        if anti:
            out[0:64, 64:128] = mat
            out[64:128, 0:64] = mat
        else:
            out[0:64, 0:64] = mat
            out[64:128, 64:128] = mat
        return out

    # wcr / wc32: rows 32q+{0..2} and 32q+{4..6}; odd q swaps column halves
    wcr = np.zeros((128, 128), np.float32)
    for q in range(4):
        lo, hi = (slice(0, 64), slice(64, 128))
        if q % 2 == 1:
            lo, hi = hi, lo
        wcr[32 * q + 0:32 * q + 3, lo] = W1xyz
        wcr[32 * q + 4:32 * q + 7, hi] = W1xyz

    R = np.linalg.pinv(W1xyz.T)                        # [3, 64]
    rt = np.zeros((128, 128), np.float32)
    for p in range(4):
        lo, hi = (slice(0, 64), slice(64, 128))
        if p % 2 == 1:
            lo, hi = hi, lo
        rt[lo, 32 * p + 0:32 * p + 3] = R.T
        rt[hi, 32 * p + 4:32 * p + 7] = R.T

    t = np.arange(N_STEPS, dtype=np.float64)
    b1t_half = np.asarray(b1, np.float64)[:, None] + np.outer(W1[3], t * DT)
    b1t = np.zeros((128, N_STEPS), np.float32)
    b1t[:64] = b1t_half
    b1t[64:] = b1t_half

    b2col = np.zeros((128, 1), np.float32)
    b2col[:64, 0] = b2
    b2col[64:, 0] = b2

    wmap = {"w2d": blk(W2, False), "w2a": blk(W2, True),
            "w13d": blk(A13, False), "w13a": blk(A13, True),
            "wcrblk": wcr, "wc32blk": wcr.copy(), "rtblk": rt,
            "b1t": b1t, "b2col": b2col}
    if use_bf16:
        npbf = mybir.dt.np(bf16)
        for k in ("w2d", "w2a", "w13d", "w13a", "wcrblk"):
            wmap[k] = wmap[k].astype(npbf)
    return wmap


V2 = True


def _kernel_v2(positions, base_velocities, W1, b1, W2, b2, W3, b3, n_steps):
    if "nc2" not in _CACHE:
        _CACHE["nc2"] = _build_nc2()
    nc = _CACHE["nc2"]

    b3 = np.asarray(b3, np.float32)
    wmap = _prep_weights2(W1, b1, W2, b2, W3, b3)
    in_maps = []
    for c in range(N_CORES):
        sl = slice(c * N_SHARD, (c + 1) * N_SHARD)
        m = _prep_core_inputs2(positions[sl], base_velocities[sl], b3)
        m.update(wmap)
        in_maps.append(m)

    res = run_bass_kernel_spmd(nc, in_maps, core_ids=list(range(N_CORES)))

    out = np.empty((N_TOTAL, 3), np.float32)
    for c in range(N_CORES):
        out[c * N_SHARD:(c + 1) * N_SHARD] = _unpack2(
            res.results[c]["out"], N_SHARD)
    return out


def kernel(positions, base_velocities, W1, b1, W2, b2, W3, b3, n_steps):
    assert int(n_steps) == N_STEPS
    positions = np.asarray(positions, np.float32)
    base_velocities = np.asarray(base_velocities, np.float32)
    if V2:
        return _kernel_v2(positions, base_velocities, W1, b1, W2, b2, W3, b3,
                          n_steps)
    W1 = np.asarray(W1, np.float32)
    b1 = np.asarray(b1, np.float32)
    W2 = np.asarray(W2, np.float32)
    b2 = np.asarray(b2, np.float32)
    W3 = np.asarray(W3, np.float32)
    b3 = np.asarray(b3, np.float32)

    if "nc" not in _CACHE:
        _CACHE["nc"] = _build_nc()
    nc = _CACHE["nc"]

    wmap = _prep_weights(W1, b1, W2, b2, W3, b3)
    in_maps = []
    for c in range(N_CORES):
        sl = slice(c * N_SHARD, (c + 1) * N_SHARD)
        m = _prep_core_inputs(positions[sl], base_velocities[sl], b3)
        m.update(wmap)
        in_maps.append(m)

    res = run_bass_kernel_spmd(nc, in_maps, core_ids=list(range(N_CORES)))

    out = np.empty((N_TOTAL, 3), np.float32)
    for c in range(N_CORES):
        out[c * N_SHARD:(c + 1) * N_SHARD] = _unpack_out(
            res.results[c]["out"], N_SHARD)
    return out



# revision 16
# speedup vs baseline: 1.1858x; 1.0300x over previous
"""Trainium2 Bass kernel for DynamicGaussianCloud (Euler integration of a
point cloud through a tiny velocity MLP, 64 steps).

The active implementation is V2 (`_build_nc2`, selected via V2=True below):
bf16 matmuls + paired 64x64 PE-tile windows + half-quad relus; see the V2
section banner for details.  The original f32r kernel (`_build_nc`) is kept
for reference/AB-testing.

V1 approach
-----------
Data-parallel over the 8 NeuronCores: each core owns N/8 = 250k points; the
MLP weights are replicated; no cross-core communication.

On-device, instead of tracking positions pos_t (3 dims/point) we track

    state_t = W1xyz.T @ pos_t            (64 dims/point, feature-major)

where W1xyz = W1[0:3, :].  Because pos_{t+1} = pos_t + DT*(W3.T@h2 + b3 + bv),
state obeys the closed recurrence

    h1_t      = relu(state_t + b1 + t*DT*W1[3,:])        (per-partition bias)
    h2_t      = relu(W2.T @ h1_t + b2)
    state_t+1 = state_t + (DT*W3@W1xyz).T @ h2_t + W1xyz.T @ bvd    (bvd = DT*(bv+b3))

so layer 1 of the MLP disappears from the loop, and the state update is pure
PSUM accumulation by the tensor engine (start=False matmuls) — no vector-engine
adds.  Positions are recovered at the end with R = pinv(W1xyz.T):
pos_T = R @ state_T (exact in infinite precision; W1 is a random Gaussian
matrix so W1xyz.T is well-conditioned).

Two 512-point blocks are packed block-diagonally on the 128 partitions
("pair" = 1024 points).  NP pairs are kept in flight (PSUM-resident state,
one bank each); emission is stage-major so the per-engine instruction
streams interleave the NP independent dependency chains.  Per pair-step:
  ACT : 1 op  (relu + per-partition bias, PSUM->SBUF)
  PE  : 3 fp32r matmuls (free dim 512 -> 1 cycle/row)
  DVE : 1 op  (fused add-bias + max0, PSUM->SBUF)
Matmuls run in float32r: fp32 storage, ~1e-4 relative multiply precision at
full 1-cycle/row speed.  Init/final matmuls run in true fp32.
"""

import sys

sys.path.insert(0, "/opt/trn_rl_repo")

import numpy as np

import concourse.bacc as bacc
import concourse.bass as bass
import concourse.mybir as mybir
import concourse.tile as tile
from concourse.bass_utils import run_bass_kernel_spmd

f32 = mybir.dt.float32
f32r = mybir.dt.float32r

N_TOTAL = 2_000_000
DT = 1.0 / 30.0
N_CORES = 8
N_SHARD = N_TOTAL // N_CORES        # 250_000
N_STEPS = 64
W = 512                             # points per block (matmul free dim)
PAIR_PTS = 2 * W                    # 1024

NP = 4                              # pairs in flight (PSUM state banks)
WBUFS = 4                           # PSUM work banks; NP + WBUFS <= 8

AluOp = mybir.AluOpType
ActFn = mybir.ActivationFunctionType

_CACHE = {}


def _layout(np_):
    """groups per core so that G*np_ pairs cover the shard."""
    g = -(-N_SHARD // (np_ * PAIR_PTS))      # ceil
    return g, g * np_ * PAIR_PTS


def _build_nc(g_count=None, n_steps=N_STEPS, repeat=1, np_=NP, wbufs=WBUFS,
              skip_mmc=False, skip_mm13=False, stagger=False, unroll=1,
              relu1_act=4, use_bf16=True):
    """Build + compile the Bass module (shapes are static)."""
    if g_count is None:
        g_count, _ = _layout(np_)
    G = g_count
    nc = bacc.Bacc("TRN2", target_bir_lowering=False, debug=False,
                   num_devices=N_CORES)

    FD = np_ * W
    pos_d = nc.declare_dram_parameter("pos", [G * 8, FD], f32, isOutput=False)
    bvd_d = nc.declare_dram_parameter("bvd", [G * 8, FD], f32r, isOutput=False)
    w2_d = nc.declare_dram_parameter("w2blk", [128, 128], f32r, isOutput=False)
    w13_d = nc.declare_dram_parameter("w13blk", [128, 128], f32r, isOutput=False)
    wc32_d = nc.declare_dram_parameter("wc32", [8, 128], f32, isOutput=False)
    wcr_d = nc.declare_dram_parameter("wcr", [8, 128], f32r, isOutput=False)
    rt_d = nc.declare_dram_parameter("rt", [128, 8], f32, isOutput=False)
    b1t_d = nc.declare_dram_parameter("b1t", [128, N_STEPS], f32, isOutput=False)
    b2_d = nc.declare_dram_parameter("b2col", [128, 1], f32, isOutput=False)
    out_d = nc.declare_dram_parameter("out", [G * 8, FD], f32, isOutput=True)

    bf16 = mybir.dt.bfloat16
    mmdt = bf16 if use_bf16 else f32r
    with tile.TileContext(nc) as tc:
        with (
            tc.tile_pool(name="const", bufs=1) as cpool,
            tc.tile_pool(name="io", bufs=2) as iopool,
            tc.tile_pool(name="hwork", bufs=2 * np_) as hpool,
            tc.tile_pool(name="fin", bufs=np_) as fpool,
            tc.tile_pool(name="state", bufs=np_, space="PSUM") as spool,
            tc.tile_pool(name="work", bufs=wbufs, space="PSUM") as wpool,
        ):
            w2_l = cpool.tile([128, 128], f32r, tag="w2l")
            w13_l = cpool.tile([128, 128], f32r, tag="w13l")
            wcr_l = cpool.tile([8, 128], f32r, tag="wcrl")
            wc32_t = cpool.tile([8, 128], f32, tag="wc32")
            rt_t = cpool.tile([128, 8], f32, tag="rt")
            b1t_t = cpool.tile([128, N_STEPS], f32, tag="b1t")
            b2_t = cpool.tile([128, 1], f32, tag="b2")
            nc.sync.dma_start(w2_l[:], w2_d[:])
            nc.sync.dma_start(w13_l[:], w13_d[:])
            nc.sync.dma_start(wc32_t[:], wc32_d[:])
            nc.sync.dma_start(wcr_l[:], wcr_d[:])
            nc.sync.dma_start(rt_t[:], rt_d[:])
            nc.sync.dma_start(b1t_t[:], b1t_d[:])
            nc.sync.dma_start(b2_t[:], b2_d[:])
            if use_bf16:
                w2_t = cpool.tile([128, 128], mmdt, tag="w2")
                w13_t = cpool.tile([128, 128], mmdt, tag="w13")
                wcr_t = cpool.tile([8, 128], mmdt, tag="wcr")
                nc.vector.tensor_copy(w2_t[:], w2_l[:])
                nc.vector.tensor_copy(w13_t[:], w13_l[:])
                nc.vector.tensor_copy(wcr_t[:], wcr_l[:])
            else:
                w2_t, w13_t, wcr_t = w2_l, w13_l, wcr_l

            assert G % unroll == 0
            with tc.For_i(0, repeat) as _r, \
                 tc.For_i(0, G // unroll, staggered_reset=stagger) as g0:
              for u in range(unroll):
                g = g0 * unroll + u
                pos_g = iopool.tile([8, FD], f32, tag="pos")
                bvd_l = iopool.tile([8, FD], f32r, tag="bvd")
                out_g = iopool.tile([8, FD], f32, tag="out")
                nc.sync.dma_start(pos_g[:], pos_d[bass.ds(g * 8, 8), :])
                nc.sync.dma_start(bvd_l[:], bvd_d[bass.ds(g * 8, 8), :])
                if use_bf16:
                    bvd_g = iopool.tile([8, FD], mmdt, tag="bvdb")
                    nc.vector.tensor_copy(bvd_g[:], bvd_l[:])
                else:
                    bvd_g = bvd_l

                # Stage-major emission: all np_ pairs per pipeline stage, so
                # each engine's instruction stream interleaves the np_
                # independent dependency chains (Tile schedules in emission
                # order per engine).
                states = []
                for p in range(np_):
                    state = spool.tile([128, W], f32, tag="state")
                    states.append(state)
                    # state_0 = W1xyz.T @ pos (true fp32, once per pair)
                    nc.tensor.matmul(state[:], wc32_t[:], pos_g[:, bass.ts(p, W)],
                                     start=True, stop=True,
                                     skip_group_check=True)
                for t in range(n_steps):
                    h1s, ps2s, h2s = [], [], []
                    for p in range(np_):
                        h1 = hpool.tile([128, W], mmdt, tag="h1")
                        h1s.append(h1)
                        if p >= relu1_act:
                            nc.vector.tensor_scalar(h1[:], states[p][:],
                                                    b1t_t[:, t:t + 1], 0.0,
                                                    AluOp.add, AluOp.max)
                        else:
                            nc.scalar.activation(h1[:], states[p][:], ActFn.Relu,
                                                 bias=b1t_t[:, t:t + 1])
                    for p in range(np_):
                        ps2 = wpool.tile([128, W], f32, tag="work")
                        ps2s.append(ps2)
                        nc.tensor.matmul(ps2[:], w2_t[:], h1s[p][:],
                                         start=True, stop=True)
                    for p in range(np_):
                        h2 = hpool.tile([128, W], mmdt, tag="h2")
                        h2s.append(h2)
                        if p >= relu1_act:
                            nc.scalar.activation(h2[:], ps2s[p][:], ActFn.Relu,
                                                 bias=b2_t[:])
                        else:
                            nc.vector.tensor_scalar(h2[:], ps2s[p][:], b2_t[:],
                                                    0.0, AluOp.add, AluOp.max)
                    for p in range(np_):
                        if not skip_mm13:
                            nc.tensor.matmul(states[p][:], w13_t[:], h2s[p][:],
                                             start=False, stop=skip_mmc,
                                             skip_group_check=True)
                        if not skip_mmc:
                            nc.tensor.matmul(states[p][:], wcr_t[:],
                                             bvd_g[:, bass.ts(p, W)],
                                             start=False, stop=True,
                                             skip_group_check=True)
                st_sbs, pos_ = [], []
                for p in range(np_):
                    st_sb = fpool.tile([128, W], f32, tag="stsb")
                    st_sbs.append(st_sb)
                    nc.scalar.activation(st_sb[:], states[p][:], ActFn.Copy)
                for p in range(np_):
                    po = wpool.tile([8, W], f32, tag="work")
                    pos_.append(po)
                    nc.tensor.matmul(po[:], rt_t[:], st_sbs[p][:],
                                     start=True, stop=True)
                for p in range(np_):
                    nc.scalar.activation(out_g[:, bass.ts(p, W)], pos_[p][:],
                                         ActFn.Copy)

                nc.sync.dma_start(out_d[bass.ds(g * 8, 8), :], out_g[:])

    nc.compile()
    return nc


def _prep_core_inputs(pos_shard, bvel_shard, b3, np_=NP):
    """Pack one core's shard into the device layout."""
    n = pos_shard.shape[0]
    G, n_pad = _layout(np_)

    def pack(arr3):
        pad = np.zeros((n_pad, 3), np.float32)
        pad[:n] = arr3
        # [G, np_, half, W, coord=4] -> rows half*4+coord, cols p*W+w
        a4 = np.zeros((G, np_, 2, W, 4), np.float32)
        a4[..., :3] = pad.reshape(G, np_, 2, W, 3)
        return a4.transpose(0, 2, 4, 1, 3).reshape(G * 8, np_ * W).copy()

    bvd = (DT * (bvel_shard + b3[None, :])).astype(np.float32)
    return {"pos": pack(pos_shard), "bvd": pack(bvd)}


def _prep_weights(W1, b1, W2, b2, W3, b3):
    W1 = np.asarray(W1, np.float64)
    W1xyz = W1[:3]                                     # [3, 64]
    W2 = np.asarray(W2, np.float64)
    W3 = np.asarray(W3, np.float64)

    w2blk = np.zeros((128, 128), np.float32)
    w2blk[:64, :64] = W2
    w2blk[64:, 64:] = W2

    lhsT13 = DT * (W3 @ W1xyz)                         # [64, 64]
    w13blk = np.zeros((128, 128), np.float32)
    w13blk[:64, :64] = lhsT13
    w13blk[64:, 64:] = lhsT13

    wc = np.zeros((8, 128), np.float32)
    wc[0:3, :64] = W1xyz
    wc[4:7, 64:] = W1xyz

    R = np.linalg.pinv(W1xyz.T)                        # [3, 64]
    rt = np.zeros((128, 8), np.float32)
    rt[:64, 0:3] = R.T
    rt[64:, 4:7] = R.T

    t = np.arange(N_STEPS, dtype=np.float64)
    b1t_half = np.asarray(b1, np.float64)[:, None] + np.outer(W1[3], t * DT)
    b1t = np.zeros((128, N_STEPS), np.float32)
    b1t[:64] = b1t_half
    b1t[64:] = b1t_half

    b2col = np.zeros((128, 1), np.float32)
    b2col[:64, 0] = b2
    b2col[64:, 0] = b2

    return {"w2blk": w2blk, "w13blk": w13blk, "wc32": wc, "wcr": wc.copy(),
            "rt": rt, "b1t": b1t, "b2col": b2col}


def _unpack_out(packed, n, np_=NP):
    """[G*8, np_*W] device layout -> [n, 3] positions."""
    G, n_pad = _layout(np_)
    a4 = packed.reshape(G, 2, 4, np_, W).transpose(0, 3, 1, 4, 2)
    return a4[..., :3].reshape(n_pad, 3)[:n]


# ======================================================================
# V2: paired-tile kernel.
#
# PE throughput on this part is capped at ~50% average utilization (activity
# throttle) — wall-clock scales with PE streaming cycles.  V2 halves them:
# the two 64x64 matmuls per pair-step (W2 and A=DT*W3@W1xyz) are issued as
# PAIRS on complementary 64x64 PE tiles: even ("diag") pairs keep state
# block-a on partitions 0-63, odd ("anti") pairs swap the halves, so
# MM2(even pair) runs on tiles (0,0)+(64,64) concurrently with MM13(odd
# pair) on tiles (0,64)+(64,0) — 2 logical matmuls per 512-cycle window.
# The per-step constant drift (wcr) runs as 4 concurrent 32x128 row-tiles
# (one per pair) in a single window.  Relus are [128, 1024] half-quad ops
# (ACT does relu1, DVE does relu2) on 2-bank PSUM slices.
# ======================================================================

bf16 = mybir.dt.bfloat16
G2 = -(-N_SHARD // (4 * PAIR_PTS))          # 62 groups of 4 pairs
N_PAD2 = G2 * 4 * PAIR_PTS


def _build_nc2(n_steps=N_STEPS, use_bf16=True, pair_mode=True):
    G = G2
    mmdt = bf16 if use_bf16 else f32r
    nc = bacc.Bacc("TRN2", target_bir_lowering=False, debug=False,
                   num_devices=N_CORES)

    pos_d = nc.declare_dram_parameter("pos", [G * 32, W], f32, isOutput=False)
    bvd_d = nc.declare_dram_parameter("bvd", [G * 32, W],
                                      mmdt if use_bf16 else f32r, isOutput=False)
    w2d_d = nc.declare_dram_parameter("w2d", [128, 128], mmdt, isOutput=False)
    w2a_d = nc.declare_dram_parameter("w2a", [128, 128], mmdt, isOutput=False)
    w13d_d = nc.declare_dram_parameter("w13d", [128, 128], mmdt, isOutput=False)
    w13a_d = nc.declare_dram_parameter("w13a", [128, 128], mmdt, isOutput=False)
    wcr_d = nc.declare_dram_parameter("wcrblk", [128, 128], mmdt, isOutput=False)
    wc32_d = nc.declare_dram_parameter("wc32blk", [128, 128], f32, isOutput=False)
    rt_d = nc.declare_dram_parameter("rtblk", [128, 128], f32, isOutput=False)
    b1t_d = nc.declare_dram_parameter("b1t", [128, N_STEPS], f32, isOutput=False)
    b2_d = nc.declare_dram_parameter("b2col", [128, 1], f32, isOutput=False)
    out_d = nc.declare_dram_parameter("out", [G * 32, W], f32, isOutput=True)

    H = 2 * W                               # half-quad relu width (2 pairs)
    FQ = 4 * W                              # full quad width

    with tile.TileContext(nc) as tc:
        with (
            tc.tile_pool(name="const", bufs=1) as cpool,
            tc.tile_pool(name="io", bufs=2) as iopool,
            tc.tile_pool(name="hwork", bufs=2) as hpool,
            tc.tile_pool(name="fin", bufs=2) as fpool,
            tc.tile_pool(name="state", bufs=1, space="PSUM") as spool,
            tc.tile_pool(name="work", bufs=1, space="PSUM") as wpool,
        ):
            w2d_t = cpool.tile([128, 128], mmdt, tag="w2d")
            w2a_t = cpool.tile([128, 128], mmdt, tag="w2a")
            w13d_t = cpool.tile([128, 128], mmdt, tag="w13d")
            w13a_t = cpool.tile([128, 128], mmdt, tag="w13a")
            wcr_t = cpool.tile([128, 128], mmdt, tag="wcr")
            wc32_t = cpool.tile([128, 128], f32, tag="wc32")
            rt_t = cpool.tile([128, 128], f32, tag="rt")
            b1t_t = cpool.tile([128, N_STEPS], f32, tag="b1t")
            b2_t = cpool.tile([128, 1], f32, tag="b2")
            nc.sync.dma_start(w2d_t[:], w2d_d[:])
            nc.sync.dma_start(w2a_t[:], w2a_d[:])
            nc.sync.dma_start(w13d_t[:], w13d_d[:])
            nc.sync.dma_start(w13a_t[:], w13a_d[:])
            nc.sync.dma_start(wcr_t[:], wcr_d[:])
            nc.sync.dma_start(wc32_t[:], wc32_d[:])
            nc.sync.dma_start(rt_t[:], rt_d[:])
            nc.sync.dma_start(b1t_t[:], b1t_d[:])
            nc.sync.dma_start(b2_t[:], b2_d[:])

            with tc.For_i(0, G) as g:
                pos_g = iopool.tile([128, W], f32, tag="pos")
                bvd_g = iopool.tile([128, W], mmdt if use_bf16 else f32r,
                                    tag="bvd")
                out_sb = iopool.tile([128, W], f32, tag="out")
                for q in range(4):
                    nc.sync.dma_start(pos_g[32 * q:32 * q + 8, :],
                                      pos_d[bass.ds(g * 32 + q * 8, 8), :])
                    nc.sync.dma_start(bvd_g[32 * q:32 * q + 8, :],
                                      bvd_d[bass.ds(g * 32 + q * 8, 8), :])

                states = spool.tile([128, FQ], f32, tag="state")
                # init (true fp32): 4 concurrent 32x128 row-tiles
                for q in range(4):
                    nc.tensor.matmul(states[:, bass.ts(q, W)],
                                     wc32_t[32 * q:32 * q + 8, :],
                                     pos_g[32 * q:32 * q + 8, :],
                                     start=True, stop=False,
                                     tile_position=(32 * q, 0),
                                     skip_group_check=True)

                def emit_mm2(ps2, h1h, p):
                    """ps2[:, p*W:] = W2blk.T @ h1; h1h = half tile, col of p within half."""
                    c = bass.ts(p, W)
                    hc = bass.ts(p % 2, W)
                    if p % 2 == 0:      # diag tiles T0 + T10
                        nc.tensor.matmul(ps2[0:64, c], w2d_t[0:64, 0:64],
                                         h1h[0:64, hc], start=True, stop=True)
                        nc.tensor.matmul(ps2[64:128, c], w2d_t[64:128, 64:128],
                                         h1h[64:128, hc], start=True, stop=True)
                    else:               # anti tiles T8 + T2
                        nc.tensor.matmul(ps2[0:64, c], w2a_t[64:128, 0:64],
                                         h1h[64:128, hc], start=True, stop=True)
                        nc.tensor.matmul(ps2[64:128, c], w2a_t[0:64, 64:128],
                                         h1h[0:64, hc], start=True, stop=True)

                def emit_mm13(states, h2h, p, stop=False):
                    """states[:, p*W:] += A13blk.T @ h2 (h2 always normal layout)."""
                    c = bass.ts(p, W)
                    hc = bass.ts(p % 2, W)
                    if p % 2 == 0:      # diag
                        nc.tensor.matmul(states[0:64, c], w13d_t[0:64, 0:64],
                                         h2h[0:64, hc], start=False, stop=stop,
                                         skip_group_check=True)
                        nc.tensor.matmul(states[64:128, c], w13d_t[64:128, 64:128],
                                         h2h[64:128, hc], start=False, stop=stop,
                                         skip_group_check=True)
                    else:               # anti: h2 block-a (S0-63) -> state[64:128]
                        nc.tensor.matmul(states[64:128, c], w13a_t[0:64, 64:128],
                                         h2h[0:64, hc], start=False, stop=stop,
                                         skip_group_check=True)
                        nc.tensor.matmul(states[0:64, c], w13a_t[64:128, 0:64],
                                         h2h[64:128, hc], start=False, stop=stop,
                                         skip_group_check=True)

                h2_prev = None          # h2 half-tile of H1 from previous step
                for t in range(n_steps):
                    # ---- half H0 (pairs 0, 1) ----
                    h1_0 = hpool.tile([128, H], mmdt, tag="h1_0")
                    nc.scalar.activation(h1_0[:], states[:, 0:H], ActFn.Relu,
                                         bias=b1t_t[:, t:t + 1])
                    ps2 = wpool.tile([128, FQ], f32, tag="ps2")
                    # W1: MM2-p0 (diag) || MM13-p3(t-1) (anti)
                    emit_mm2(ps2, h1_0, 0)
                    if h2_prev is not None:
                        emit_mm13(states, h2_prev, 3)
                    # W2: MM2-p1 (anti) || MM13-p2(t-1) (diag)
                    emit_mm2(ps2, h1_0, 1)
                    if h2_prev is not None:
                        emit_mm13(states, h2_prev, 2)
                    # One 4-tile wcr window here, off the chain tail: q0/q1
                    # fire as soon as W2 drains (their relu1-H0 WAR is long
                    # satisfied) and stream while relu2-H0 runs on DVE; q2/q3
                    # wait only for relu1-H1 (ACT, done well before relu2-H0).
                    # Keeping all 4 tiles in one window holds mode switches
                    # at 2/step — splitting them measured +1.4ms.
                    for q in range(4):
                        nc.tensor.matmul(states[:, bass.ts(q, W)],
                                         wcr_t[32 * q:32 * q + 8, :],
                                         bvd_g[32 * q:32 * q + 8, :],
                                         start=False, stop=True,
                                         tile_position=(32 * q, 0),
                                         skip_group_check=True)
                    h2_0 = hpool.tile([128, H], mmdt, tag="h2_0")
                    nc.vector.tensor_scalar(h2_0[:], ps2[:, 0:H], b2_t[:], 0.0,
                                            AluOp.add, AluOp.max)
                    # ---- half H1 (pairs 2, 3) ----
                    h1_1 = hpool.tile([128, H], mmdt, tag="h1_1")
                    nc.scalar.activation(h1_1[:], states[:, H:FQ], ActFn.Relu,
                                         bias=b1t_t[:, t:t + 1])
                    # W3: MM13-p1(t) (anti) || MM13-p0(t) (diag) — both MM13s
                    # first, so relu1-H0(t+1) waits only for this window, not
                    # for the MM2s of pairs 2/3.
                    emit_mm13(states, h2_0, 1)
                    emit_mm13(states, h2_0, 0)
                    # W4: MM2-p2 (diag) || MM2-p3 (anti)
                    emit_mm2(ps2, h1_1, 2)
                    emit_mm2(ps2, h1_1, 3)
                    h2_1 = hpool.tile([128, H], mmdt, tag="h2_1")
                    nc.vector.tensor_scalar(h2_1[:], ps2[:, H:FQ], b2_t[:], 0.0,
                                            AluOp.add, AluOp.max)
                    h2_prev = h2_1

                # tail: MM13-p2(T-1) (diag) || MM13-p3(T-1) (anti)
                emit_mm13(states, h2_prev, 2, stop=True)
                emit_mm13(states, h2_prev, 3, stop=True)

                # finale
                st_sb = fpool.tile([128, FQ], f32, tag="stsb")
                nc.scalar.activation(st_sb[:], states[:], ActFn.Copy)
                out_ps = wpool.tile([128, FQ], f32, tag="ps2")
                for p in range(4):
                    nc.tensor.matmul(out_ps[32 * p:32 * p + 8, 0:W],
                                     rt_t[:, 32 * p:32 * p + 8],
                                     st_sb[:, bass.ts(p, W)],
                                     start=True, stop=True,
                                     tile_position=(0, 32 * p),
                                     skip_group_check=True)
                nc.scalar.activation(out_sb[:], out_ps[:, 0:W], ActFn.Copy)
                for q in range(4):
                    nc.sync.dma_start(out_d[bass.ds(g * 32 + q * 8, 8), :],
                                      out_sb[32 * q:32 * q + 8, :])

    nc.compile()
    return nc


def _pack2(arr3, n):
    """[n,3] -> [G2*32, W]: row = g*32 + q*8 + half*4 + coord, col = point."""
    pad = np.zeros((N_PAD2, 3), np.float32)
    pad[:n] = arr3
    a5 = pad.reshape(G2, 4, 2, W, 3)              # g, q, half, w, coord
    buf = np.zeros((G2, 4, 2, 4, W), np.float32)  # g, q, half, coord, w
    buf[:, :, :, :3, :] = a5.transpose(0, 1, 2, 4, 3)
    return buf.reshape(G2 * 32, W)


def _unpack2(packed, n):
    a5 = packed.reshape(G2, 4, 2, 4, W)[:, :, :, :3, :]
    return a5.transpose(0, 1, 2, 4, 3).reshape(N_PAD2, 3)[:n].copy()


def _prep_core_inputs2(pos_shard, bvel_shard, b3, use_bf16=True):
    n = pos_shard.shape[0]
    bvd = (DT * (bvel_shard + b3[None, :])).astype(np.float32)
    bvd_p = _pack2(bvd, n)
    if use_bf16:
        bvd_p = bvd_p.astype(mybir.dt.np(bf16))
    return {"pos": _pack2(pos_shard, n), "bvd": bvd_p}


def _prep_weights2(W1, b1, W2, b2, W3, b3, use_bf16=True):
    W1 = np.asarray(W1, np.float64)
    W1xyz = W1[:3]                                     # [3, 64]
    W2 = np.asarray(W2, np.float64)
    W3 = np.asarray(W3, np.float64)
    A13 = DT * (W3 @ W1xyz)                            # [64, 64] lhsT

    def blk(mat, anti):
        out = np.zeros((128, 128), np.float32)
        if anti:
            out[0:64, 64:128] = mat
            out[64:128, 0:64] = mat
        else:
            out[0:64, 0:64] = mat
            out[64:128, 64:128] = mat
        return out

    # wcr / wc32: rows 32q+{0..2} and 32q+{4..6}; odd q swaps column halves
    wcr = np.zeros((128, 128), np.float32)
    for q in range(4):
        lo, hi = (slice(0, 64), slice(64, 128))
        if q % 2 == 1:
            lo, hi = hi, lo
        wcr[32 * q + 0:32 * q + 3, lo] = W1xyz
        wcr[32 * q + 4:32 * q + 7, hi] = W1xyz

    R = np.linalg.pinv(W1xyz.T)                        # [3, 64]
    rt = np.zeros((128, 128), np.float32)
    for p in range(4):
        lo, hi = (slice(0, 64), slice(64, 128))
        if p % 2 == 1:
            lo, hi = hi, lo
        rt[lo, 32 * p + 0:32 * p + 3] = R.T
        rt[hi, 32 * p + 4:32 * p + 7] = R.T

    t = np.arange(N_STEPS, dtype=np.float64)
    b1t_half = np.asarray(b1, np.float64)[:, None] + np.outer(W1[3], t * DT)
    b1t = np.zeros((128, N_STEPS), np.float32)
    b1t[:64] = b1t_half
    b1t[64:] = b1t_half

    b2col = np.zeros((128, 1), np.float32)
    b2col[:64, 0] = b2
    b2col[64:, 0] = b2

    wmap = {"w2d": blk(W2, False), "w2a": blk(W2, True),
            "w13d": blk(A13, False), "w13a": blk(A13, True),
            "wcrblk": wcr, "wc32blk": wcr.copy(), "rtblk": rt,
            "b1t": b1t, "b2col": b2col}
    if use_bf16:
        npbf = mybir.dt.np(bf16)
        for k in ("w2d", "w2a", "w13d", "w13a", "wcrblk"):
            wmap[k] = wmap[k].astype(npbf)
    return wmap


V2 = True


def _kernel_v2(positions, base_velocities, W1, b1, W2, b2, W3, b3, n_steps):
    if "nc2" not in _CACHE:
        _CACHE["nc2"] = _build_nc2()
    nc = _CACHE["nc2"]

    b3 = np.asarray(b3, np.float32)
    wmap = _prep_weights2(W1, b1, W2, b2, W3, b3)
    in_maps = []
    for c in range(N_CORES):
        sl = slice(c * N_SHARD, (c + 1) * N_SHARD)
        m = _prep_core_inputs2(positions[sl], base_velocities[sl], b3)
        m.update(wmap)
        in_maps.append(m)

    res = run_bass_kernel_spmd(nc, in_maps, core_ids=list(range(N_CORES)))

    out = np.empty((N_TOTAL, 3), np.float32)
    for c in range(N_CORES):
        out[c * N_SHARD:(c + 1) * N_SHARD] = _unpack2(
            res.results[c]["out"], N_SHARD)
    return out


def kernel(positions, base_velocities, W1, b1, W2, b2, W3, b3, n_steps):
    assert int(n_steps) == N_STEPS
    positions = np.asarray(positions, np.float32)
    base_velocities = np.asarray(base_velocities, np.float32)
    if V2:
        return _kernel_v2(positions, base_velocities, W1, b1, W2, b2, W3, b3,
                          n_steps)
    W1 = np.asarray(W1, np.float32)
    b1 = np.asarray(b1, np.float32)
    W2 = np.asarray(W2, np.float32)
    b2 = np.asarray(b2, np.float32)
    W3 = np.asarray(W3, np.float32)
    b3 = np.asarray(b3, np.float32)

    if "nc" not in _CACHE:
        _CACHE["nc"] = _build_nc()
    nc = _CACHE["nc"]

    wmap = _prep_weights(W1, b1, W2, b2, W3, b3)
    in_maps = []
    for c in range(N_CORES):
        sl = slice(c * N_SHARD, (c + 1) * N_SHARD)
        m = _prep_core_inputs(positions[sl], base_velocities[sl], b3)
        m.update(wmap)
        in_maps.append(m)

    res = run_bass_kernel_spmd(nc, in_maps, core_ids=list(range(N_CORES)))

    out = np.empty((N_TOTAL, 3), np.float32)
    for c in range(N_CORES):
        out[c * N_SHARD:(c + 1) * N_SHARD] = _unpack_out(
            res.results[c]["out"], N_SHARD)
    return out

